# revision 1
# baseline (speedup 1.0000x reference)
"""ComplexLayerNorm Trainium2 kernel (8 NeuronCores, SPMD, C-sharded).

Math (see reference): per-feature 2x2 covariance whitening of (re, im) over
all B*C samples (centered with the batch-only mean mu_b), after subtracting
the complex mean over F, plus complex affine.

Sharding: C (=128) split 16-per-core, so the batch-sums T[c,f] = sum_b x and
mu_b are core-local and only 3 partial second-moment F-vectors (24 KB) need an
AllReduce.

v2 restructure (vs the fp32-matmul baseline):
  * The host feeds x already transposed to the f-on-partitions layout
    (xT[p, 1024*t + j] = x[j, 128*t + p]) -- host-side data marshaling like
    the pre-tiled gamma/beta.  This deletes the PE transpose matmuls and the
    PSUM->SBUF copies entirely; the kernel streams 16 f-chunks per component
    and every stat overlaps the DMA stream:
      - ACT: Square w/ accum for S_rr, S_ii and S_(r+i)^2 (the cross moment
        comes from the polarization identity, avoiding a DVE pass)
      - Pool: the (x_r + x_i) sum feeding the third Square
      - DVE: T[c,f] batch-sums as strided tensor_reduce over b
      - PE: mean-over-F ones-matmuls (float32r, accumulated across chunks)
  * All matmuls (mean, correction, apply) use float32r bitcast views:
    1 cyc/row at output free size >= 256 vs fp32's 4.
Per-core dataflow otherwise identical to the baseline:
  stats -> AllReduce (24 KB) -> closed-form 2x2 inverse sqrt w/ gamma folded
  -> PE apply with sparse-diagonal W producing the interleaved (f, 2) output.
"""

import numpy as np

import bass_rust
import concourse.bass as bass
import concourse.mybir as mybir
from concourse import tile
from concourse.bass_utils import run_bass_kernel_spmd


def split_multi_waits(nc):
    """The walrus build in this container allows only ONE sync-wait command
    per instruction; Tile emits several.  Split extras into preceding
    single-wait NoOps on the same engine (sequential waits == AND)."""
    cnt = 0
    for bb in nc.main_func.blocks:
        il = bb.instructions
        newlist = []
        changed = False
        for inst in list(il):
            si = inst.sync_info
            waits = list(si.on_wait) if si else []
            if len(waits) > 1:
                changed = True
                for w in waits[:-1]:
                    cnt += 1
                    nop = bass_rust.InstNoOp(name=f"I-wsplit-{cnt}")
                    nop.engine = inst.engine
                    nop.sync_info = mybir.SyncInfo(on_wait=[w], on_update=[])
                    newlist.append(nop)
                inst.sync_info = mybir.SyncInfo(
                    on_wait=[waits[-1]], on_update=list(si.on_update))
            newlist.append(inst)
        if changed:
            il[:] = newlist
    return cnt

FP = mybir.dt.float32
FR = mybir.dt.float32r
BF = mybir.dt.bfloat16
AF = mybir.ActivationFunctionType
OP = mybir.AluOpType
AX = mybir.AxisListType

B, C, F = 64, 128, 2048
NCORES = 8
CSH = C // NCORES           # 16 channels per core
BC = B * CSH                # 1024 sample rows per core
NFT = F // 128              # 16 f-chunks
NBB = BC // 128             # 8 bc-blocks
EPS = 1e-4
NM1 = float(B * C - 1)      # 8191


def build_bass():
    nc = bass.Bass()

    # host-pre-transposed: xt[p, 1024*t + j] = x[j, 128*t + p]
    xt_r = nc.dram_tensor("xt_r", [128, NFT * BC], FR, kind="ExternalInput")
    xt_i = nc.dram_tensor("xt_i", [128, NFT * BC], FR, kind="ExternalInput")
    # gamma pre-tiled on host to (128, NFT): tile[p, t] = gamma[128*t + p]
    g_r = nc.dram_tensor("g_r", [128, NFT], FP, kind="ExternalInput")
    g_i = nc.dram_tensor("g_i", [128, NFT], FP, kind="ExternalInput")
    # beta interleaved on host: (1, 4096) = [b_r[0], b_i[0], b_r[1], ...]
    beta_ilv = nc.dram_tensor("beta_ilv", [1, 2 * F], FR, kind="ExternalInput")
    ident = nc.dram_tensor("ident", [128, 128], FP, kind="ExternalInput")
    onesF = nc.dram_tensor("onesF", [128, 1], FR, kind="ExternalInput")
    ones1 = nc.dram_tensor("ones1", [1, 128], FR, kind="ExternalInput")

    out = nc.dram_tensor("out", [BC, 2 * F], BF, kind="ExternalOutput")

    with tile.TileContext(nc) as tc:
        with (
            tc.tile_pool(name="big", bufs=1) as big,
            tc.tile_pool(name="small", bufs=1) as small,
            tc.tile_pool(name="wpool", bufs=2) as wpool,
            tc.tile_pool(name="stage", bufs=2) as stage,
            tc.tile_pool(name="dram", bufs=1, space="DRAM") as dram,
        ):
            # ---- constants to SBUF
            ident_t = small.tile([128, 128], FP, tag="ident")
            nc.sync.dma_start(ident_t[:], ident[:])
            onesF_t = small.tile([128, 1], FR, tag="onesF")
            nc.sync.dma_start(onesF_t[:], onesF[:])
            ones1_t = small.tile([1, 128], FR, tag="ones1")
            nc.sync.dma_start(ones1_t[:], ones1[:])
            beta_sb = small.tile([1, 2 * F], FR, tag="beta_sb")
            g_r_t = small.tile([128, NFT], FP, tag="g_r")
            nc.sync.dma_start(g_r_t[:], g_r[:])
            g_i_t = small.tile([128, NFT], FP, tag="g_i")
            nc.sync.dma_start(g_i_t[:], g_i[:])

            # ---- persistent: x transposed, xT[p, 1024*t + j] = x[j, 128*t+p]
            xT_r = big.tile([128, NFT * BC], FR, tag="xT_r")
            xT_i = big.tile([128, NFT * BC], FR, tag="xT_i")

            # T batch-sums: T_sb[p, 16*t + c] = sum_b x[b*16+c, 128*t+p]
            T_r_sb = small.tile([128, NFT * CSH], FP, tag="T_r_sb")
            T_i_sb = small.tile([128, NFT * CSH], FP, tag="T_i_sb")

            # per-f second moments, one column per f-chunk
            S_rr = small.tile([128, NFT], FP, tag="S_rr")
            S_ri = small.tile([128, NFT], FP, tag="S_ri")
            S_ii = small.tile([128, NFT], FP, tag="S_ii")

            from contextlib import ExitStack
            _stk = ExitStack()
            scratch = _stk.enter_context(tc.tile_pool(name="scratch", bufs=1))
            scr2 = _stk.enter_context(tc.tile_pool(name="scr2", bufs=2))
            ps_mean = _stk.enter_context(
                tc.tile_pool(name="ps_mean", bufs=1, space="PSUM"))

            # ---- Phase A: stream f-chunks; stats + mean overlap the DMA
            psm_r = ps_mean.tile([1, BC], FP, tag="psm_r")
            psm_i = ps_mean.tile([1, BC], FP, tag="psm_i")
            for t in range(NFT):
                sl = slice(BC * t, BC * (t + 1))
                nc.sync.dma_start(xT_r[:, sl], xt_r[:, sl])
                nc.sync.dma_start(xT_i[:, sl], xt_i[:, sl])
                # mean-over-F contribution (PE, accumulated across chunks)
                for xT, psm in ((xT_r, psm_r), (xT_i, psm_i)):
                    for h in range(2):
                        nc.tensor.matmul(
                            psm[:, 512 * h:512 * (h + 1)],
                            onesF_t[:],
                            xT[:, BC * t + 512 * h:BC * t + 512 * (h + 1)],
                            start=(t == 0), stop=(t == NFT - 1),
                        )
                # second moments: S_rr/S_ii on ACT, S_ri on DVE
                sca = scratch.tile([128, BC], FP, tag="sq_act")
                nc.scalar.activation(sca[:], xT_r[:, sl].bitcast(FP), AF.Square,
                                     accum_out=S_rr[:, t:t + 1])
                sca2 = scratch.tile([128, BC], FP, tag="sq_act")
                nc.scalar.activation(sca2[:], xT_i[:, sl].bitcast(FP), AF.Square,
                                     accum_out=S_ii[:, t:t + 1])
                scd = scratch.tile([128, BC], FP, tag="sq_dve")
                nc.vector.scalar_tensor_tensor(
                    out=scd[:], in0=xT_r[:, sl].bitcast(FP), scalar=1.0,
                    in1=xT_i[:, sl].bitcast(FP), op0=OP.mult, op1=OP.mult,
                    accum_out=S_ri[:, t:t + 1],
                )
                # T batch-sums: two Pool tree-add stages over b, DVE tail
                for xT, T_sb in ((xT_r, T_r_sb), (xT_i, T_i_sb)):
                    xv = xT[:, sl].bitcast(FP).rearrange(
                        "p (b two c) -> p b two c", two=2, c=CSH)
                    tr1 = scr2.tile([128, B * CSH // 2], FP, tag="tr1")
                    t1w = tr1[:].rearrange("p (b c) -> p b c", c=CSH)
                    nc.gpsimd.tensor_tensor(out=t1w, in0=xv[:, :, 0, :],
                                            in1=xv[:, :, 1, :], op=OP.add)
                    t1v = tr1[:].rearrange("p (b two c) -> p b two c",
                                           two=2, c=CSH)
                    tr2 = scr2.tile([128, B * CSH // 4], FP, tag="tr2")
                    t2w = tr2[:].rearrange("p (b c) -> p b c", c=CSH)
                    nc.gpsimd.tensor_tensor(out=t2w, in0=t1v[:, :, 0, :],
                                            in1=t1v[:, :, 1, :], op=OP.add)
                    nc.vector.tensor_reduce(
                        T_sb[:, CSH * t:CSH * (t + 1)],
                        tr2[:].rearrange("p (b c) -> p c b", c=CSH),
                        AX.X, OP.add,
                    )

            # ---- T quadratic correction: corr_xy[:, t] = sum_c T_x*T_y
            corr_rr = small.tile([128, NFT], FP, tag="corr_rr")
            corr_ri = small.tile([128, NFT], FP, tag="corr_ri")
            corr_ii = small.tile([128, NFT], FP, tag="corr_ii")
            for corr, (Ta, Tb) in (
                (corr_rr, (T_r_sb, T_r_sb)),
                (corr_ri, (T_r_sb, T_i_sb)),
                (corr_ii, (T_i_sb, T_i_sb)),
            ):
                prod = scratch.tile([128, NFT * CSH], FP, tag="tprod",
                                    name=f"tprod_{corr.tensor.name}")
                nc.vector.scalar_tensor_tensor(
                    out=prod[:], in0=Ta[:], scalar=1.0, in1=Tb[:],
                    op0=OP.mult, op1=OP.mult,
                )
                nc.vector.tensor_reduce(
                    corr[:],
                    prod[:].rearrange("p (t c) -> p t c", c=CSH),
                    AX.X, OP.add,
                )

            # ---- local partial covariance: (S - corr/B) / (n-1), packed
            partial = small.tile([128, 3 * NFT], FP, tag="partial")
            for j, (S, corr) in enumerate(
                ((S_rr, corr_rr), (S_ri, corr_ri), (S_ii, corr_ii))
            ):
                dst = partial[:, NFT * j:NFT * (j + 1)]
                nc.vector.scalar_tensor_tensor(
                    out=dst, in0=corr[:], scalar=-1.0 / B, in1=S[:],
                    op0=OP.mult, op1=OP.add,
                )
                nc.vector.tensor_scalar(
                    out=dst, in0=dst, scalar1=1.0 / NM1, scalar2=None,
                    op0=OP.mult,
                )

            # ---- AllReduce partial covariance (24 KB)
            ar_in = dram.tile([128, 3 * NFT], FP, tag="ar_in")
            ar_out = dram.tile([128, 3 * NFT], FP, tag="ar_out")
            nc.sync.dma_start(ar_in[:], partial[:])
            nc.gpsimd.collective_compute(
                "AllReduce", OP.add,
                replica_groups=[list(range(NCORES))],
                ins=[ar_in.opt()],
                outs=[ar_out.opt()],
            )
            cov = small.tile([128, 3 * NFT], FP, tag="cov")
            nc.sync.dma_start(cov[:], ar_out[:])
            nc.vector.memset(beta_sb[0:1, 0:128].bitcast(FP), 0.0)
            nc.scalar.dma_start(beta_sb[:], beta_ilv[:])

            # ---- mean broadcast + in-place centering of xT (overlaps AR)
            from contextlib import ExitStack as _ES2
            _stk2 = _ES2()
            ps_mb = _stk2.enter_context(
                tc.tile_pool(name="ps_mb", bufs=1, space="PSUM"))
            Mb = {}
            for comp, psm in (("r", psm_r), ("i", psm_i)):
                mrow = small.tile([1, BC], FR, tag=f"mrow_{comp}")
                nc.vector.tensor_copy(mrow[:], psm[:])
                pmb = ps_mb.tile([128, BC], FP, tag=f"pmb_{comp}")
                for h in range(2):
                    nc.tensor.matmul(
                        pmb[:, 512 * h:512 * (h + 1)],
                        ones1_t[:],
                        mrow[0:1, 512 * h:512 * (h + 1)],
                        start=True, stop=True,
                    )
                mb = small.tile([128, BC], FP, tag=f"Mb_{comp}")
                nc.vector.tensor_copy(mb[:], pmb[:])
                Mb[comp] = mb
            for t in range(NFT):
                sl = slice(BC * t, BC * (t + 1))
                for comp, xT in (("r", xT_r), ("i", xT_i)):
                    idx = 2 * t + (comp == "i")
                    eng = nc.vector if idx % 3 == 2 else nc.gpsimd
                    eng.tensor_tensor(
                        out=xT[:, sl], in0=xT[:, sl].bitcast(FP),
                        in1=Mb[comp][:], op=OP.subtract,
                    )
            _stk2.close()
            # release phase-A pools (scratch SBUF, mean PSUM)
            _stk.close()

            # ---- Phase C: closed-form 2x2 inverse sqrt, fold gamma -> A
            def stile(tag):
                return small.tile([128, NFT], FP, tag=tag, name=tag)

            arr, bri, cii = stile("arr"), stile("bri"), stile("cii")
            nc.vector.tensor_scalar(out=arr[:], in0=cov[:, 0:NFT],
                                    scalar1=EPS, scalar2=None, op0=OP.add)
            nc.vector.tensor_copy(bri[:], cov[:, NFT:2 * NFT])
            nc.vector.tensor_scalar(out=cii[:], in0=cov[:, 2 * NFT:3 * NFT],
                                    scalar1=EPS, scalar2=None, op0=OP.add)

            det, tmp = stile("det"), stile("tmp")
            nc.vector.tensor_tensor(out=det[:], in0=arr[:], in1=cii[:],
                                    op=OP.mult)
            nc.vector.tensor_tensor(out=tmp[:], in0=bri[:], in1=bri[:],
                                    op=OP.mult)
            nc.vector.tensor_tensor(out=det[:], in0=det[:], in1=tmp[:],
                                    op=OP.subtract)
            s_t = stile("s_t")
            nc.scalar.activation(s_t[:], det[:], AF.Sqrt)
            # tval = sqrt(a + c + 2 s)
            tsum = stile("tsum")
            nc.vector.tensor_tensor(out=tsum[:], in0=arr[:], in1=cii[:],
                                    op=OP.add)
            nc.vector.scalar_tensor_tensor(out=tsum[:], in0=s_t[:], scalar=2.0,
                                           in1=tsum[:], op0=OP.mult, op1=OP.add)
            tval = stile("tval")
            nc.scalar.activation(tval[:], tsum[:], AF.Sqrt)
            den, rden = stile("den"), stile("rden")
            nc.vector.tensor_tensor(out=den[:], in0=s_t[:], in1=tval[:],
                                    op=OP.mult)
            nc.vector.reciprocal(rden[:], den[:])

            w_rr, w_ii, wri_n = stile("w_rr"), stile("w_ii"), stile("wri_n")
            # w_rr = (c+s)*rden ; w_ii = (a+s)*rden ; w_ri = -b*rden = wri_n
            nc.vector.tensor_tensor(out=w_rr[:], in0=cii[:], in1=s_t[:],
                                    op=OP.add)
            nc.vector.tensor_tensor(out=w_rr[:], in0=w_rr[:], in1=rden[:],
                                    op=OP.mult)
            nc.vector.tensor_tensor(out=w_ii[:], in0=arr[:], in1=s_t[:],
                                    op=OP.add)
            nc.vector.tensor_tensor(out=w_ii[:], in0=w_ii[:], in1=rden[:],
                                    op=OP.mult)
            nc.vector.tensor_tensor(out=wri_n[:], in0=bri[:], in1=rden[:],
                                    op=OP.mult)
            nc.vector.tensor_scalar(out=wri_n[:], in0=wri_n[:], scalar1=-1.0,
                                    scalar2=None, op0=OP.mult)

            # A = G @ W,  G = [[g_r, -g_i], [g_i, g_r]], W = [[w_rr, w_ri],
            # [w_ri, w_ii]] with w_ri = wri_n
            a_rr, a_ri = stile("a_rr"), stile("a_ri")
            a_ir, a_ii = stile("a_ir"), stile("a_ii")
            u, v = stile("u"), stile("v")
            # a_rr = g_r*w_rr - g_i*w_ri
            nc.vector.tensor_tensor(out=u[:], in0=g_r_t[:], in1=w_rr[:],
                                    op=OP.mult)
            nc.vector.tensor_tensor(out=v[:], in0=g_i_t[:], in1=wri_n[:],
                                    op=OP.mult)
            nc.vector.tensor_tensor(out=a_rr[:], in0=u[:], in1=v[:],
                                    op=OP.subtract)
            # a_ri = g_r*w_ri - g_i*w_ii
            nc.vector.tensor_tensor(out=u[:], in0=g_r_t[:], in1=wri_n[:],
                                    op=OP.mult)
            nc.vector.tensor_tensor(out=v[:], in0=g_i_t[:], in1=w_ii[:],
                                    op=OP.mult)
            nc.vector.tensor_tensor(out=a_ri[:], in0=u[:], in1=v[:],
                                    op=OP.subtract)
            # a_ir = g_i*w_rr + g_r*w_ri
            nc.vector.tensor_tensor(out=u[:], in0=g_i_t[:], in1=w_rr[:],
                                    op=OP.mult)
            nc.vector.tensor_tensor(out=v[:], in0=g_r_t[:], in1=wri_n[:],
                                    op=OP.mult)
            nc.vector.tensor_tensor(out=a_ir[:], in0=u[:], in1=v[:],
                                    op=OP.add)
            # a_ii = g_i*w_ri + g_r*w_ii
            nc.vector.tensor_tensor(out=u[:], in0=g_i_t[:], in1=wri_n[:],
                                    op=OP.mult)
            nc.vector.tensor_tensor(out=v[:], in0=g_r_t[:], in1=w_ii[:],
                                    op=OP.mult)
            nc.vector.tensor_tensor(out=a_ii[:], in0=u[:], in1=v[:],
                                    op=OP.add)

            # ---- Phase D: apply.  t-outer; W built on the fly.
            _stk3 = ExitStack()
            ps_o = _stk3.enter_context(
                tc.tile_pool(name="ps_o", bufs=4, space="PSUM"))
            for t2 in range(NFT // 2):
                ta, tb = 2 * t2, 2 * t2 + 1
                Ws = []
                for t in (ta, tb):
                    W_r = wpool.tile([128, 256], FR, tag="W_r",
                                     name=f"W_r_{t}")
                    W_i = wpool.tile([128, 256], FR, tag="W_i",
                                     name=f"W_i_{t}")
                    for W, (ev, od) in ((W_r, (a_rr, a_ir)),
                                        (W_i, (a_ri, a_ii))):
                        Wv = W[:].rearrange("p (g c) -> p g c", c=2)
                        nc.vector.tensor_scalar(
                            out=Wv[:, :, 0], in0=ident_t[:],
                            scalar1=ev[:, t:t + 1], scalar2=None, op0=OP.mult,
                        )
                        nc.vector.tensor_scalar(
                            out=Wv[:, :, 1], in0=ident_t[:],
                            scalar1=od[:, t:t + 1], scalar2=None, op0=OP.mult,
                        )
                    Ws.append((W_r, W_i))
                for bh in range(2):
                    stg = stage.tile([128, 4 * 512], BF, tag="stg")
                    for bb in range(4):
                        b = 4 * bh + bb
                        po = ps_o.tile([128, 512], FP, tag="po")
                        # one accumulation group: beta first (start=True
                        # over the full tile), then the centered-x terms.
                        nc.tensor.matmul(
                            po[:],
                            ones1_t[:],
                            beta_sb[0:1, 512 * t2:512 * (t2 + 1)],
                            start=True, stop=False,
                        )
                        for j, t in enumerate((ta, tb)):
                            W_r, W_i = Ws[j]
                            sl = slice(BC * t + 128 * b,
                                       BC * t + 128 * (b + 1))
                            nc.tensor.matmul(
                                po[:, 256 * j:256 * (j + 1)],
                                xT_r[:, sl], W_r[:],
                                start=False, stop=False,
                            )
                            nc.tensor.matmul(
                                po[:, 256 * j:256 * (j + 1)],
                                xT_i[:, sl], W_i[:],
                                start=False, stop=(j == 1),
                            )
                        if b % 2 == 0:
                            nc.vector.tensor_copy(
                                stg[:, 512 * bb:512 * (bb + 1)], po[:])
                        else:
                            nc.scalar.copy(
                                stg[:, 512 * bb:512 * (bb + 1)], po[:])
                    # 1 MB store: rows (b, p) -> out[128*b + p, 512*t2:+512]
                    # alternate stores across the two HWDGE rings
                    dst = out.rearrange("(a p) f -> p a f", p=128)[
                        :, 4 * bh:4 * (bh + 1), 512 * t2:512 * (t2 + 1)
                    ]
                    src = stg[:].rearrange("p (a q) -> p a q", q=512)
                    if (2 * t2 + bh) % 2 == 0:
                        nc.sync.dma_start(dst, src)
                    else:
                        nc.scalar.dma_start(dst, src)
            _stk3.close()

    split_multi_waits(nc)
    return nc


_CACHE = {}


def _get_nc():
    if "nc" not in _CACHE:
        _CACHE["nc"] = build_bass()
    return _CACHE["nc"]


def _constants():
    if "consts" not in _CACHE:
        _CACHE["consts"] = {
            "ident": np.eye(128, dtype=np.float32),
            "onesF": np.full((128, 1), 1.0 / F, dtype=np.float32),
            "ones1": np.ones((1, 128), dtype=np.float32),
        }
    return _CACHE["consts"]


def _host_transpose(x):
    """(BC, F) -> xt[p, 1024*t + j] = x[j, 128*t + p] as (128, NFT*BC)."""
    xs = x.reshape(BC, NFT, 128)              # (j, t, p)
    return np.ascontiguousarray(
        np.transpose(xs, (2, 1, 0)).reshape(128, NFT * BC))


def kernel(x_real, x_imag, gamma_r, gamma_i, beta_r, beta_i):
    x_real = np.ascontiguousarray(x_real, dtype=np.float32)
    x_imag = np.ascontiguousarray(x_imag, dtype=np.float32)
    gamma_r = np.asarray(gamma_r, dtype=np.float32)
    gamma_i = np.asarray(gamma_i, dtype=np.float32)
    beta_r = np.asarray(beta_r, dtype=np.float32)
    beta_i = np.asarray(beta_i, dtype=np.float32)

    nc = _get_nc()
    consts = _constants()
    g_r_t = np.ascontiguousarray(gamma_r.reshape(NFT, 128).T)
    g_i_t = np.ascontiguousarray(gamma_i.reshape(NFT, 128).T)
    beta_ilv = np.ascontiguousarray(
        np.stack([beta_r, beta_i], axis=-1).reshape(1, 2 * F)
    )

    in_maps = []
    for k in range(NCORES):
        cs = slice(CSH * k, CSH * (k + 1))
        in_maps.append({
            "xt_r": _host_transpose(x_real[:, cs, :].reshape(BC, F)),
            "xt_i": _host_transpose(x_imag[:, cs, :].reshape(BC, F)),
            "g_r": g_r_t, "g_i": g_i_t, "beta_ilv": beta_ilv,
            **consts,
        })

    res = run_bass_kernel_spmd(nc, in_maps, list(range(NCORES)))

    full = np.empty((B, C, F, 2), dtype=np.float32)
    for k in range(NCORES):
        full[:, CSH * k:CSH * (k + 1)] = (
            np.asarray(res.results[k]["out"]).astype(np.float32)
            .reshape(B, CSH, F, 2)
        )
    return full

